# revision 5
# baseline (speedup 1.0000x reference)
"""Trainium2 Bass kernel for nn_DynamicMLP (3-layer LIF spiking net, T=16).

Strategy (8 NeuronCores, data-parallel over batch):
  - Shard batch 1024 -> 8 x 128. Replicate weights. Zero cross-core comms.
  - Layout: [batch=128 partitions, hidden on free dim].
  - The LIF current c lives in PSUM scaled by 2^t:
      C_t = sum_{tau<=t} 2^tau * I_tau  ==  2^t * (c_t - bias part)
    Bias is factored out of the recursion (fixed point):
      c_t = 2^-t * C_t + (2 - 2^-t) * b
  - fp32-exact matmuls via fp16 hi/lo splits (fp16 x fp16 exact in fp32 PSUM):
      L0 (x inexact in fp16): 3 passes  xh@wh -> C0; xl@wh, xh@(wl*2^11) -> C0b
      L1/L2 (spikes exact):   2 passes  s@wh -> C;  (s*2^-11)@(wl*2^11) -> C
  - LIF algebra refactored to minimize the post-matmul critical chain:
      v_t = v0^2 - 0.172*U0 + c_t         (U == u / 0.172)
      U_t = 1.529*U0 - v0 ; U_{t+1} = U_t + (0.132/0.172)*s_t
    r = v0^2 - 0.172*U0 and r2 = r + (2-2^-t)*b precompute BEFORE C is ready,
    so the chain after the last matmul is just stt(C) -> spike -> transpose.
  - Elementwise work is spread over DVE / ACT / Pool engines; the PE runs only
    the 144 GEMM instructions per step (the precision-mandated minimum).
  - Host packs x and W hi/lo interleaved so every DMA row is a >=512B run.
"""
import sys

sys.path.insert(0, "/opt/trn_rl_repo")

import numpy as np

import concourse.bacc as bacc
import concourse.tile as tile
from concourse import mybir
from concourse.bass_utils import run_bass_kernel_spmd

dt = mybir.dt
F16 = dt.float16
F32 = dt.float32
Alu = mybir.AluOpType

NCORES = 8
FULL = dict(T=16, IN=2048, H0=1024, H1=1024, OUT=512, BL=128)
TH_V = -0.172
TH_U = 0.529
TH_S = 0.132
C_RESET = 0.021

_BUILD_CACHE = {}


def build(T=16, IN=2048, H0=1024, H1=1024, OUT=512, BL=128):
    key = (T, IN, H0, H1, OUT, BL)
    if key in _BUILD_CACHE:
        return _BUILD_CACHE[key]
    KT0, KT1, KT2 = IN // 128, H0 // 128, H1 // 128
    KH = KT0 // 2          # ktiles per x chunk
    NCH = 512              # psum bank free-dim (fp32)
    HS = {0: H0, 1: H1, 2: OUT}
    KTS = {1: KT1, 2: KT2}
    HTOT = H0 + H1 + OUT
    BOFF = {0: 0, 1: H0, 2: H0 + H1}

    nc = bacc.Bacc("TRN2", target_bir_lowering=False, debug=False, num_devices=NCORES)

    xp_d = nc.dram_tensor("xp", [T, IN, 2 * BL], F16, kind="ExternalInput")
    w_d = {"w0": nc.dram_tensor("w0p", [IN, 2 * H0], F16, kind="ExternalInput"),
           "w1": nc.dram_tensor("w1p", [H0, 2 * H1], F16, kind="ExternalInput"),
           "wo": nc.dram_tensor("wop", [H1, 2 * OUT], F16, kind="ExternalInput")}
    b_d = nc.dram_tensor("brep2", [128, HTOT], F32, kind="ExternalInput")
    nb_d = nc.dram_tensor("negb", [2, HTOT], F16, kind="ExternalInput")
    o2_d = nc.dram_tensor("ones2", [2, 128], F16, kind="ExternalInput")
    out_d = nc.dram_tensor("out", [BL, OUT], F32, kind="ExternalOutput")

    with tile.TileContext(nc) as tc:
        with tc.tile_pool(name="w", bufs=1) as wp, \
             tc.tile_pool(name="state", bufs=1) as sp, \
             tc.tile_pool(name="xs", bufs=2) as xp, \
             tc.tile_pool(name="spk", bufs=2) as kp, \
             tc.tile_pool(name="psum", bufs=1, space="PSUM") as pp:

            # ---- resident weight tiles (DMAs issued later, first-use order) --
            w_sb = {"w0": [wp.tile([128, 2 * H0], F16, tag=f"w0_{k}", name=f"w0_{k}")
                           for k in range(KT0)],
                    "w1": [wp.tile([128, 2 * H1], F16, tag=f"w1_{k}", name=f"w1_{k}")
                           for k in range(KT1)],
                    "wo": [wp.tile([128, 2 * OUT], F16, tag=f"wo_{k}", name=f"wo_{k}")
                           for k in range(KT2)]}

            def dma_w(nm, ks):
                for k in ks:
                    nc.sync.dma_start(out=w_sb[nm][k][:],
                                      in_=w_d[nm][k * 128:(k + 1) * 128, :])

            bfull = wp.tile([128, HTOT], F32, tag="bfull", name="bfull")  # 2*b
            negb = wp.tile([2, HTOT], F16, tag="negb", name="negb")
            ones2 = wp.tile([2, 128], F16, tag="ones2", name="ones2")

            # ---- states ----
            vm = {l: sp.tile([128, HS[l]], F32, tag=f"vm{l}", name=f"vm{l}")
                  for l in range(3)}
            vr = {l: sp.tile([128, HS[l]], F32, tag=f"vr{l}", name=f"vr{l}")
                  for l in range(3)}
            U = {l: sp.tile([128, HS[l]], F32, tag=f"U{l}", name=f"U{l}")
                 for l in range(3)}
            r2 = {l: sp.tile([128, HS[l]], F32, tag=f"r2{l}", name=f"r2{l}")
                  for l in range(3)}
            r = sp.tile([128, max(H0, H1)], F32, tag="r", name="r")
            c021 = sp.tile([128, max(H0, H1)], F32, tag="c021", name="c021")
            acc = sp.tile([128, OUT], F32, tag="acc", name="acc")

            C = {0: pp.tile([128, H0], F32, tag="C0", name="C0"),
                 1: pp.tile([128, H1], F32, tag="C1", name="C1"),
                 2: pp.tile([128, OUT], F32, tag="C2", name="C2")}
            C0b = pp.tile([128, H0], F32, tag="C0b", name="C0b")

            # ---- x tile loads (2 chunks per step; 512B dram runs) ----
            x_pre = {}

            def load_x(t):
                tiles = []
                for ci in range(2):
                    xt = xp.tile([128, KH * 2 * BL], F16, tag=f"x{ci}",
                                 name=f"x_t{t}_{ci}")
                    ks = ci * KH * 128
                    nc.sync.dma_start(
                        out=xt[:].rearrange("p (k b) -> p k b", b=2 * BL),
                        in_=xp_d[t:t + 1, ks:ks + KH * 128, :].rearrange(
                            "o (k p) b -> p (o k) b", p=128))
                    tiles.append(xt)
                x_pre[t] = tiles

            # ---- matmul emitters ----
            def emit_L0(t, ci):
                xt = x_pre[t][ci]
                if ci == 1:
                    x_pre.pop(t, None)
                for k in range(KH):
                    kg = ci * KH + k
                    la = xt[:, k * 256:k * 256 + 128]
                    lr = xt[:, k * 256 + 128:(k + 1) * 256]
                    wt = w_sb["w0"][kg]
                    for n0 in range(0, H0, NCH):
                        first = (t == 0 and kg == 0)
                        last = (t == T - 1 and kg == KT0 - 1)
                        ra = wt[:, n0:n0 + NCH]
                        rl = wt[:, H0 + n0:H0 + n0 + NCH]
                        nc.tensor.matmul(C[0][:, n0:n0 + NCH], la, ra, start=False,
                                         stop=last, skip_group_check=True)
                        psb = C0b[:, n0:n0 + NCH]
                        nc.tensor.matmul(psb, lr, ra, start=first, stop=False,
                                         skip_group_check=True)
                        nc.tensor.matmul(psb, la, rl, start=False, stop=last,
                                         skip_group_check=True)

            def emit_L(l, t, sT, sL):
                h = HS[l]
                for k in range(KTS[l]):
                    la = sT[:, k * 128:(k + 1) * 128]
                    lr = sL[:, k * 128:(k + 1) * 128]
                    wt = w_sb["w1" if l == 1 else "wo"][k]
                    for n0 in range(0, h, NCH):
                        nn = min(NCH, h - n0)
                        last = (t == T - 1 and k == KTS[l] - 1)
                        ps = C[l][:, n0:n0 + nn]
                        nc.tensor.matmul(ps, la, wt[:, n0:n0 + nn], start=False,
                                         stop=False, skip_group_check=True)
                        nc.tensor.matmul(ps, lr, wt[:, h + n0:h + n0 + nn],
                                         start=False, stop=last,
                                         skip_group_check=True)

            def emit_bias_init():
                """Seed C0/C1/C2 with -b via a K=2 matmul (rows 1, 2^-11).
                Then c_t = 2^-t*C_t + 2b for all t."""
                for l in range(3):
                    h = HS[l]
                    for n0 in range(0, h, NCH):
                        nn = min(NCH, h - n0)
                        nc.tensor.matmul(
                            C[l][:, n0:n0 + nn], ones2[:],
                            negb[:, BOFF[l] + n0:BOFF[l] + n0 + nn],
                            start=True, stop=False, skip_group_check=True)

            # ---- LIF pieces ----
            def emit_E(l, t):
                """Precompute r2_l = vr^2 - 0.172*U + 2b and U_t = 1.529*U - vr.
                Runs before C[l] is ready."""
                h = HS[l]
                nc.scalar.square(r[:, :h], vr[l][:])
                nc.vector.scalar_tensor_tensor(
                    out=r[:, :h], in0=U[l][:], scalar=TH_V, in1=r[:, :h],
                    op0=Alu.mult, op1=Alu.add)
                nc.gpsimd.tensor_tensor(
                    out=r2[l][:], in0=bfull[:, BOFF[l]:BOFF[l] + h],
                    in1=r[:, :h], op=Alu.add)
                nc.gpsimd.tensor_scalar(out=U[l][:], in0=U[l][:],
                                        scalar1=float(1.0 + TH_U), scalar2=None,
                                        op0=Alu.mult)
                nc.gpsimd.tensor_tensor(out=U[l][:], in0=U[l][:], in1=vr[l][:],
                                        op=Alu.subtract)

            def vhead(l, t):
                """v_t = 2^-t*C (+2^-(t+11)*C0b) + r2 — frees C[l] for t+1."""
                if l == 0:
                    nc.vector.scalar_tensor_tensor(
                        out=vm[0][:], in0=C0b[:], scalar=float(2.0 ** -(t + 11)),
                        in1=r2[0][:], op0=Alu.mult, op1=Alu.add)
                    nc.vector.scalar_tensor_tensor(
                        out=vm[0][:], in0=C[0][:], scalar=float(2.0 ** -t),
                        in1=vm[0][:], op0=Alu.mult, op1=Alu.add)
                else:
                    nc.vector.scalar_tensor_tensor(
                        out=vm[l][:], in0=C[l][:], scalar=float(2.0 ** -t),
                        in1=r2[l][:], op0=Alu.mult, op1=Alu.add)

            def vrest(l, t):
                """spike + state updates + E(l, t+1); returns (sT, sL) for l<2."""
                h = HS[l]
                last = (t == T - 1)
                if l == 2:
                    s2 = kp.tile([128, OUT], F32, tag="s2", name=f"s2_t{t}",
                                 bufs=1)
                    nc.vector.tensor_scalar(out=s2[:], in0=vm[2][:], scalar1=0.5,
                                            scalar2=1.0, op0=Alu.is_gt,
                                            op1=Alu.mult)
                    nc.gpsimd.tensor_tensor(out=acc[:], in0=acc[:], in1=s2[:],
                                            op=Alu.add)
                    if not last:
                        nc.vector.scalar_tensor_tensor(
                            out=U[2][:], in0=s2[:], scalar=float(TH_S / -TH_V),
                            in1=U[2][:], op0=Alu.mult, op1=Alu.add)
                        nc.scalar.copy(vr[2][:], vm[2][:])
                        nc.vector.copy_predicated(
                            out=vr[2][:], mask=s2[:].bitcast(dt.uint32),
                            data=c021[:, :h])
                        emit_E(2, t + 1)
                    return None, None
                sc = float(2.0 ** t)
                s = kp.tile([128, h], F16, tag="s", name=f"s{l}_t{t}")
                nc.vector.tensor_scalar(out=s[:], in0=vm[l][:], scalar1=0.5,
                                        scalar2=sc, op0=Alu.is_gt, op1=Alu.mult)
                sT = kp.tile([128, h], F16, tag="sT", name=f"sT{l}_t{t}")
                nc.scalar.dma_start_transpose(
                    out=sT[:].rearrange("p (k b) -> p k b", b=128), in_=s[:])
                sL = kp.tile([128, h], F16, tag="s", name=f"sL{l}_t{t}")
                nc.scalar.mul(sL[:], sT[:], float(2.0 ** -11))
                if not last:
                    nc.vector.scalar_tensor_tensor(
                        out=U[l][:], in0=s[:], scalar=float(TH_S / -TH_V / sc),
                        in1=U[l][:], op0=Alu.mult, op1=Alu.add)
                    nc.scalar.copy(vr[l][:], vm[l][:])
                    nc.vector.copy_predicated(
                        out=vr[l][:], mask=s[:].bitcast(dt.uint16),
                        data=c021[:, :h])
                    emit_E(l, t + 1)
                return sT, sL

            # ---- preamble: DMAs in first-use order + state init ----
            nc.sync.dma_start(out=ones2[:], in_=o2_d[:])
            nc.sync.dma_start(out=negb[:], in_=nb_d[:])
            emit_bias_init()
            load_x(0)
            dma_w("w0", range(0, KH))
            nc.sync.dma_start(out=bfull[:, :H0], in_=b_d[:, :H0])
            nc.vector.memset(c021[:], C_RESET)
            nc.gpsimd.memset(acc[:], 0.0)
            for l in range(3):
                nc.vector.memset(vr[l][:], 0.0)
                nc.gpsimd.memset(U[l][:], 0.0)
            dma_w("w0", range(KH, KT0))
            nc.sync.dma_start(out=bfull[:, H0:], in_=b_d[:, H0:])
            load_x(1)
            dma_w("w1", range(KT1))
            dma_w("wo", range(KT2))
            # t=0 precompute: states are zero, so r2 = 2b and U_mid = 0
            for l in range(3):
                nc.vector.tensor_copy(out=r2[l][:],
                                      in_=bfull[:, BOFF[l]:BOFF[l] + HS[l]])

            # ---- steady state: 1-step layer skew ----
            def emit_rest(t):
                sT0, sL0 = vrest(0, t)
                emit_L(1, t, sT0, sL0)
                if t + 1 < T:
                    emit_L0(t + 1, 1)
                vhead(1, t)
                sT1, sL1 = vrest(1, t)
                emit_L(2, t, sT1, sL1)
                vhead(2, t)
                vrest(2, t)

            for t in range(T):
                if t >= 1:
                    vhead(0, t - 1)
                emit_L0(t, 0)
                if t >= 1:
                    emit_rest(t - 1)
                else:
                    emit_L0(0, 1)
                if t + 2 < T:
                    load_x(t + 2)
            # ---- final step: chunked spike chains + hi-then-lo GEMMs to cut
            # the PE's wait on s0T/s1T during the pipeline drain ----
            def final_chain(l, t):
                h = HS[l]
                sc = float(2.0 ** t)
                s = kp.tile([128, h], F16, tag="s", name=f"s{l}_t{t}")
                sT = kp.tile([128, h], F16, tag="sT", name=f"sT{l}_t{t}")
                FC = 256
                for c in range(h // FC):
                    sl = slice(c * FC, (c + 1) * FC)
                    if l == 0:
                        nc.vector.scalar_tensor_tensor(
                            out=vm[0][:, sl], in0=C0b[:, sl],
                            scalar=float(2.0 ** -(t + 11)), in1=r2[0][:, sl],
                            op0=Alu.mult, op1=Alu.add)
                        nc.vector.scalar_tensor_tensor(
                            out=vm[0][:, sl], in0=C[0][:, sl],
                            scalar=float(2.0 ** -t), in1=vm[0][:, sl],
                            op0=Alu.mult, op1=Alu.add)
                    else:
                        nc.vector.scalar_tensor_tensor(
                            out=vm[l][:, sl], in0=C[l][:, sl],
                            scalar=float(2.0 ** -t), in1=r2[l][:, sl],
                            op0=Alu.mult, op1=Alu.add)
                    nc.vector.tensor_scalar(out=s[:, sl], in0=vm[l][:, sl],
                                            scalar1=0.5, scalar2=sc,
                                            op0=Alu.is_gt, op1=Alu.mult)
                    kt = FC // 128
                    nc.scalar.dma_start_transpose(
                        out=sT[:].rearrange("p (k b) -> p k b", b=128)
                            [:, c * kt:(c + 1) * kt, :],
                        in_=s[:, sl])
                sL = kp.tile([128, h], F16, tag="s", name=f"sL{l}_t{t}")
                nc.scalar.mul(sL[:], sT[:], float(2.0 ** -11))
                return sT, sL

            def emit_L_hilo(l, t, sT, sL):
                h = HS[l]
                wtag = {1: "w1", 2: "wo"}[l]
                for k in range(KTS[l]):
                    la = sT[:, k * 128:(k + 1) * 128]
                    wt = w_sb[wtag][k]
                    for n0 in range(0, h, NCH):
                        nn = min(NCH, h - n0)
                        nc.tensor.matmul(C[l][:, n0:n0 + nn], la,
                                         wt[:, n0:n0 + nn], start=False,
                                         stop=False, skip_group_check=True)
                for k in range(KTS[l]):
                    lr = sL[:, k * 128:(k + 1) * 128]
                    wt = w_sb[wtag][k]
                    for n0 in range(0, h, NCH):
                        nn = min(NCH, h - n0)
                        nc.tensor.matmul(C[l][:, n0:n0 + nn], lr,
                                         wt[:, h + n0:h + n0 + nn], start=False,
                                         stop=(t == T - 1 and k == KTS[l] - 1),
                                         skip_group_check=True)

            u = T - 1
            sT0, sL0 = final_chain(0, u)
            emit_L_hilo(1, u, sT0, sL0)
            sT1, sL1 = final_chain(1, u)
            emit_L_hilo(2, u, sT1, sL1)
            # final layer-2 chain straight on DVE, then output DMA
            sfin = kp.tile([128, OUT], F32, tag="s2", name="s2_final", bufs=1)
            for c0 in range(0, OUT, 256):
                sl = slice(c0, c0 + 256)
                nc.vector.scalar_tensor_tensor(
                    out=vm[2][:, sl], in0=C[2][:, sl], scalar=float(2.0 ** -u),
                    in1=r2[2][:, sl], op0=Alu.mult, op1=Alu.add)
                nc.vector.tensor_scalar(out=sfin[:, sl], in0=vm[2][:, sl],
                                        scalar1=0.5, scalar2=1.0,
                                        op0=Alu.is_gt, op1=Alu.mult)
                nc.vector.scalar_tensor_tensor(
                    out=acc[:, sl], in0=sfin[:, sl], scalar=1.0,
                    in1=acc[:, sl], op0=Alu.mult, op1=Alu.add)
                nc.sync.dma_start(out=out_d[:, sl], in_=acc[:, sl])

    nc.compile()
    _BUILD_CACHE[key] = nc
    return nc


def prep_inputs(in_pop_spikes, W0, b0, W1, b1, Wout, bout,
                T=16, BL=128, ncores=NCORES):
    """Host-side prep: pack hi/lo-split x and W with interleaved layout."""
    x = np.ascontiguousarray(np.transpose(np.asarray(in_pop_spikes, np.float32),
                                          (2, 1, 0)))  # [T, IN, B]
    B = x.shape[2]
    scale = (2.0 ** np.arange(T, dtype=np.float32)).reshape(T, 1, 1)
    xh32 = x.astype(np.float16).astype(np.float32)
    xa = (xh32 * scale).astype(np.float16)                 # exact 2^t * fp16(x)
    xr = ((x - xh32) * (scale * np.float32(2048.0))).astype(np.float16)

    com = {}
    for nm, W in (("w0p", W0), ("w1p", W1), ("wop", Wout)):
        WT = np.asarray(W, np.float32).T
        hi = WT.astype(np.float16)
        lo = ((WT - hi.astype(np.float32)) * np.float32(2048.0)).astype(np.float16)
        com[nm] = np.ascontiguousarray(
            np.concatenate([hi, lo], axis=1))            # [K, 2*H]
    b = np.concatenate([np.asarray(v, np.float32) for v in (b0, b1, bout)])
    com["brep2"] = np.ascontiguousarray(
        np.broadcast_to(np.float32(2.0) * b, (128, b.size)))
    bh = b.astype(np.float16)
    bl = ((b - bh.astype(np.float32)) * np.float32(2048.0)).astype(np.float16)
    com["negb"] = np.ascontiguousarray(np.stack([-bh, -bl]))
    o2 = np.zeros((2, 128), np.float16)
    o2[0] = 1.0
    o2[1] = np.float16(2.0 ** -11)
    com["ones2"] = o2

    in_maps = []
    for c in range(ncores):
        m = dict(com)
        xs = np.stack([xa[:, :, c * BL:(c + 1) * BL],
                       xr[:, :, c * BL:(c + 1) * BL]], axis=2)  # [T, IN, 2, BL]
        m["xp"] = np.ascontiguousarray(xs.reshape(T, xs.shape[1], 2 * BL))
        in_maps.append(m)
    return in_maps


def kernel(in_pop_spikes, W0, b0, W1, b1, Wout, bout, batch_size, _trace=False):
    T = in_pop_spikes.shape[2]
    nc = build(**FULL)
    in_maps = prep_inputs(in_pop_spikes, W0, b0, W1, b1, Wout, bout, T=T)
    res = run_bass_kernel_spmd(nc, in_maps, core_ids=list(range(NCORES)),
                               trace=_trace)
    out = np.concatenate([r["out"] for r in res.results], axis=0)
    out = (out / np.float32(T)).astype(np.float32)
    if _trace:
        kernel._last_results = res
    return out
